# revision 14
# baseline (speedup 1.0000x reference)
# Trainium2 Bass kernel for nn_AttnSeqTimeDecayModel (retrieval_knn).
#
# Strategy (8 NeuronCores, history dim N=200000 sharded 25000/core):
#   per core: alpha = vs_shard @ v via PE (vs pre-transposed on host so the
#   contraction dim lands on partitions); per-partition-row top-16 candidates
#   (max8+match_replace); AllGather the 8x[128,16] candidate values; every
#   core computes the exact global threshold T = 51st largest via the gpsimd
#   kth_largest instruction on the gathered union; selection = alpha > T
#   (exactly 50 winners, verified margins); winners' hs rows + ts gathered via
#   indirect DMA from an augmented [25000,192] table; per-core partial
#   softmax-numerator/denominator; second AllGather combines partials; every
#   core finishes attention + score head + GRU; host reads core 0's outputs.
import numpy as np

TOPIC = 128
HID = 128
K = 50
N_HIST = 200000
NCORES = 8
SHARD = N_HIST // NCORES          # 25000
COLS = 196                        # ceil(SHARD/128)
PADN = COLS * 128                 # 25088
NEG = -1.0e30
ZROW = float(SHARD)               # index of the all-zero row in hs_aug
AUGN = SHARD + 8                  # hs_aug rows (includes zero rows)
LN_DECAY = float(np.log1p(-1e-07))  # ln(1 - 1e-7), computed in float64
AUGW = 192                        # hs_aug row width (768B, multiple of 256B)
QUANT = 1.0 - 49.5 / (NCORES * 128 * 16 - 1)  # k_adj = 49 -> out desc[50]

_CACHE = {}


def _build_program():
    import concourse.bacc as bacc
    import concourse.bass as bass
    import concourse.mybir as mybir
    import concourse.tile as tile
    from concourse import library_config

    dt = mybir.dt
    f32 = dt.float32

    nc = bacc.Bacc(
        "TRN2",
        target_bir_lowering=False,
        debug=False,
        num_devices=NCORES,
    )

    # ---- kernel I/O (per-core DRAM tensors) ----
    vsT_d = nc.dram_tensor("vsT", [128, PADN], f32, kind="ExternalInput")
    hsaug_d = nc.dram_tensor("hs_aug", [AUGN, AUGW], f32, kind="ExternalInput")
    vcol_d = nc.dram_tensor("vcol", [128, 1], f32, kind="ExternalInput")
    scal_d = nc.dram_tensor("scal", [1, 8], f32, kind="ExternalInput")
    ws1_d = nc.dram_tensor("Ws1", [128, 1], f32, kind="ExternalInput")
    ws2_d = nc.dram_tensor("Ws2", [128, 1], f32, kind="ExternalInput")
    wihT_d = nc.dram_tensor("WihT", [128, 384], f32, kind="ExternalInput")
    wihs_d = nc.dram_tensor("Wihs", [128, 3], f32, kind="ExternalInput")
    whhT_d = nc.dram_tensor("WhhT", [128, 384], f32, kind="ExternalInput")
    bih3_d = nc.dram_tensor("bih3", [128, 3], f32, kind="ExternalInput")
    bhh3_d = nc.dram_tensor("bhh3", [128, 3], f32, kind="ExternalInput")
    h0_d = nc.dram_tensor("h0", [128, 1], f32, kind="ExternalInput")
    iota_d = nc.dram_tensor("iota_p", [128, 1], f32, kind="ExternalInput")

    score_d = nc.dram_tensor("score_out", [1, 1], f32, kind="ExternalOutput")
    hout_d = nc.dram_tensor("h_out", [128, 1], f32, kind="ExternalOutput")
    dbg_d = nc.dram_tensor("dbg_out", [128, 32], f32, kind="ExternalOutput")

    groups = [list(range(NCORES))]
    NCHUNK = 7
    CCOLS = COLS // NCHUNK            # 28 matmul tiles per chunk
    CW = CCOLS * 128                  # 3584 elements per chunk per partition

    with tile.TileContext(nc) as tc:
        with (
            tc.tile_pool(name="sb", bufs=1) as sb,
            tc.tile_pool(name="ps", bufs=1, space="PSUM") as ps,
            tc.tile_pool(name="dram", bufs=1, space="DRAM") as dram,
        ):
            # Q7 ucode library for kth_largest; issue first so the reload
            # overlaps the streaming phase.
            nc.gpsimd.load_library(library_config.attn)

            # ---- small loads ----
            v_sb = sb.tile([128, 1], f32, tag="v_sb")
            nc.sync.dma_start(v_sb[:], vcol_d[:])
            scal_sb = sb.tile([1, 8], f32, tag="scal_sb")
            nc.sync.dma_start(scal_sb[:], scal_d[:])
            ws1_sb = sb.tile([128, 1], f32, tag="ws1_sb")
            nc.sync.dma_start(ws1_sb[:], ws1_d[:])
            ws2_sb = sb.tile([128, 1], f32, tag="ws2_sb")
            nc.sync.dma_start(ws2_sb[:], ws2_d[:])
            wihT_sb = sb.tile([128, 384], f32, tag="wihT_sb")
            nc.sync.dma_start(wihT_sb[:], wihT_d[:])
            wihs_sb = sb.tile([128, 3], f32, tag="wihs_sb")
            nc.sync.dma_start(wihs_sb[:], wihs_d[:])
            whhT_sb = sb.tile([128, 384], f32, tag="whhT_sb")
            nc.sync.dma_start(whhT_sb[:], whhT_d[:])
            bih3_sb = sb.tile([128, 3], f32, tag="bih3_sb")
            nc.sync.dma_start(bih3_sb[:], bih3_d[:])
            bhh3_sb = sb.tile([128, 3], f32, tag="bhh3_sb")
            nc.sync.dma_start(bhh3_sb[:], bhh3_d[:])
            h0_sb = sb.tile([128, 1], f32, tag="h0_sb")
            nc.sync.dma_start(h0_sb[:], h0_d[:])
            iota_sb = sb.tile([128, 1], f32, tag="iota_sb")
            nc.sync.dma_start(iota_sb[:], iota_d[:])

            ones_row = sb.tile([1, 128], f32, tag="ones_row")
            nc.vector.memset(ones_row[:], 1.0)
            ones_col = sb.tile([128, 1], f32, tag="ones_col")
            nc.vector.memset(ones_col[:], 1.0)

            # ---- alpha = vs @ v, laid out [128, 196]: alpha[p, j] = a[j*128+p]
            alpha_ps = ps.tile([128, COLS], f32, tag="alpha_ps")
            chunks = []
            for i in range(NCHUNK):
                ch = sb.tile([128, CW], f32, tag=f"vs_chunk{i}")
                nc.sync.dma_start(ch[:], vsT_d[:, i * CW:(i + 1) * CW])
                chunks.append(ch)
            for i in range(NCHUNK):
                for j in range(CCOLS):
                    col = i * CCOLS + j
                    nc.tensor.matmul(
                        alpha_ps[:, col:col + 1],
                        lhsT=chunks[i][:, j * 128:(j + 1) * 128],
                        rhs=v_sb[:],
                        start=True,
                        stop=True,
                    )

            # padded vsT columns are built host-side so pad alphas ~= -1e9
            alpha_sb = sb.tile([128, COLS], f32, tag="alpha_sb")
            nc.scalar.copy(out=alpha_sb[:], in_=alpha_ps[:])

            # ---- per-row top-16 candidates ----
            cand = sb.tile([128, 16], f32, tag="cand")
            work = sb.tile([128, COLS], f32, tag="work")
            nc.vector.max(out=cand[:, 0:8], in_=alpha_sb[:])
            nc.vector.match_replace(
                out=work[:], in_to_replace=cand[:, 0:8], in_values=alpha_sb[:],
                imm_value=NEG,
            )
            nc.vector.max(out=cand[:, 8:16], in_=work[:])

            # ---- AllGather candidates; exact global threshold T ----
            cand_dr = dram.tile([128, 16], f32, tag="cand_dr")
            ag1_out = dram.tile([NCORES * 128, 16], f32, tag="ag1_out")
            nc.sync.dma_start(cand_dr[:], cand[:])
            nc.gpsimd.collective_compute(
                "AllGather",
                mybir.AluOpType.bypass,
                replica_groups=groups,
                ins=[cand_dr[:]],
                outs=[ag1_out[:]],
            )
            union = sb.tile([128, 128], f32, tag="union")
            for c in range(NCORES):
                nc.sync.dma_start(
                    union[:, c * 16:(c + 1) * 16],
                    ag1_out[c * 128:(c + 1) * 128, :],
                )
            kth = sb.tile([1, 2], f32, tag="kth")
            nc.gpsimd.kth_largest(
                kth[:], union[:], n_per_lane=128, k=64, quantile=QUANT,
            )

            # broadcast (T, t, s) to all partitions via K=1 matmul
            small = sb.tile([1, 3], f32, tag="small")
            nc.vector.tensor_copy(out=small[:, 0:1], in_=kth[:, 1:2])
            nc.vector.tensor_copy(out=small[:, 1:2], in_=scal_sb[:, 0:1])
            nc.vector.tensor_copy(out=small[:, 2:3], in_=scal_sb[:, 1:2])
            bc_ps = ps.tile([128, 3], f32, tag="bc_ps")
            nc.tensor.matmul(bc_ps[:], lhsT=ones_row[:], rhs=small[:],
                             start=True, stop=True)
            bcast = sb.tile([128, 3], f32, tag="bcast")
            nc.scalar.copy(out=bcast[:], in_=bc_ps[:])
            negT = sb.tile([128, 1], f32, tag="negT")
            nc.vector.tensor_scalar_mul(negT[:], bcast[:, 0:1], -1.0)

            # ---- selection: per-row top8 values/indices, mask vs T ----
            m8v = sb.tile([128, 8], f32, tag="m8v")
            m8i = sb.tile([128, 8], dt.uint32, tag="m8i")
            nc.vector.max(out=m8v[:], in_=alpha_sb[:])
            nc.vector.max_index(out=m8i[:], in_max=m8v[:], in_values=alpha_sb[:])
            m8if = sb.tile([128, 8], f32, tag="m8if")
            nc.scalar.copy(out=m8if[:], in_=m8i[:])
            li = sb.tile([128, 8], f32, tag="li")
            # li = m8if * 128 + p  (local row index)
            nc.vector.scalar_tensor_tensor(
                out=li[:], in0=m8if[:], scalar=128.0,
                in1=iota_sb[:].to_broadcast([128, 8]),
                op0=mybir.AluOpType.mult, op1=mybir.AluOpType.add,
            )
            mask = sb.tile([128, 8], f32, tag="mask")
            nc.vector.tensor_tensor(
                out=mask[:], in0=m8v[:],
                in1=bcast[:, 0:1].to_broadcast([128, 8]),
                op=mybir.AluOpType.is_gt,
            )
            # sel = mask ? li : ZROW   (as (li - ZROW)*mask + ZROW); all
            # indices stay in-bounds — unselected slots hit the zero row.
            sel = sb.tile([128, 8], f32, tag="sel")
            nc.vector.scalar_tensor_tensor(
                out=sel[:], in0=li[:], scalar=ZROW, in1=mask[:],
                op0=mybir.AluOpType.subtract, op1=mybir.AluOpType.mult,
            )
            nc.vector.tensor_scalar_add(sel[:], sel[:], ZROW)
            sel_i = sb.tile([128, 8], dt.int32, tag="sel_i")
            nc.scalar.copy(out=sel_i[:], in_=sel[:])

            # ---- gather winners' (h row, ts) via indirect DMA ----
            # one gather per candidate rank, offsets [128, 1] (one per
            # partition) — the layout tile_scatter_add uses on HW.
            hsel = sb.tile([128, 8, AUGW], f32, tag="hsel")
            nc.vector.memset(hsel[:], 0.0)
            for j in range(8):
                nc.gpsimd.indirect_dma_start(
                    out=hsel[:, j, :],
                    out_offset=None,
                    in_=hsaug_d[:],
                    in_offset=bass.IndirectOffsetOnAxis(
                        ap=sel_i[:, j:j + 1], axis=0
                    ),
                )

            # ---- decayed softmax weights ----
            ts_sel = hsel[:, :, HID:HID + 1]        # [128, 8, 1]
            delta = sb.tile([128, 8], f32, tag="delta")
            nc.vector.tensor_tensor(
                out=delta[:],
                in0=bcast[:, 1:2].to_broadcast([128, 8]),
                in1=ts_sel,
                op=mybir.AluOpType.subtract,
            )
            decay = sb.tile([128, 8], f32, tag="decay")
            nc.scalar.activation(
                decay[:], delta[:], mybir.ActivationFunctionType.Exp,
                scale=LN_DECAY,
            )
            a_d = sb.tile([128, 8], f32, tag="a_d")
            nc.vector.tensor_mul(out=a_d[:], in0=m8v[:], in1=decay[:])
            ex = sb.tile([128, 8], f32, tag="ex")
            nc.scalar.activation(
                ex[:], a_d[:], mybir.ActivationFunctionType.Exp,
                bias=negT[:], scale=1.0,
            )
            u = sb.tile([128, 8], f32, tag="u")
            nc.vector.tensor_mul(out=u[:], in0=ex[:], in1=mask[:])

            # ---- per-core partial numerator / denominator ----
            num_ps = ps.tile([128, 1], f32, tag="num_ps")
            for j in range(8):
                nc.tensor.matmul(
                    num_ps[:],
                    lhsT=hsel[:, j, 0:HID],
                    rhs=u[:, j:j + 1],
                    start=(j == 0),
                    stop=(j == 7),
                )
            urs = sb.tile([128, 1], f32, tag="urs")
            nc.vector.reduce_sum(out=urs[:], in_=u[:], axis=mybir.AxisListType.X)
            den_ps = ps.tile([1, 1], f32, tag="den_ps")
            nc.tensor.matmul(den_ps[:], lhsT=urs[:], rhs=ones_col[:],
                             start=True, stop=True)
            num_sb = sb.tile([128, 1], f32, tag="num_sb")
            nc.scalar.copy(out=num_sb[:], in_=num_ps[:])
            den_pack = sb.tile([1, 8], f32, tag="den_pack")
            nc.vector.memset(den_pack[:], 0.0)
            nc.scalar.copy(out=den_pack[:, 0:1], in_=den_ps[:])

            # ---- AllGather partials, reduce ----
            part_dr = dram.tile([1, 136], f32, tag="part_dr")
            ag2_out = dram.tile([NCORES, 136], f32, tag="ag2_out")
            nc.sync.dma_start(part_dr[0:1, 0:128], num_sb[:])
            nc.sync.dma_start(part_dr[0:1, 128:136], den_pack[:])
            nc.gpsimd.collective_compute(
                "AllGather",
                mybir.AluOpType.bypass,
                replica_groups=groups,
                ins=[part_dr[:]],
                outs=[ag2_out[:]],
            )
            nums = sb.tile([128, NCORES], f32, tag="nums")
            for c in range(NCORES):
                nc.sync.dma_start(nums[:, c:c + 1], ag2_out[c:c + 1, 0:128])
            dens = sb.tile([1, NCORES], f32, tag="dens")
            nc.sync.dma_start(dens[:], ag2_out[:, 128:129])

            num_tot = sb.tile([128, 1], f32, tag="num_tot")
            nc.vector.reduce_sum(out=num_tot[:], in_=nums[:],
                                 axis=mybir.AxisListType.X)
            den_tot = sb.tile([1, 1], f32, tag="den_tot")
            nc.vector.reduce_sum(out=den_tot[:], in_=dens[:],
                                 axis=mybir.AxisListType.X)
            rden = sb.tile([1, 1], f32, tag="rden")
            nc.vector.reciprocal(out=rden[:], in_=den_tot[:])
            rb_ps = ps.tile([128, 1], f32, tag="rb_ps")
            nc.tensor.matmul(rb_ps[:], lhsT=ones_row[:], rhs=rden[:],
                             start=True, stop=True)
            attn = sb.tile([128, 1], f32, tag="attn")
            nc.vector.tensor_mul(out=attn[:], in0=num_tot[:], in1=rb_ps[:])

            # ---- score head ----
            s_ps = ps.tile([1, 1], f32, tag="s_ps")
            nc.tensor.matmul(s_ps[:], lhsT=v_sb[:], rhs=ws1_sb[:],
                             start=True, stop=False)
            nc.tensor.matmul(s_ps[:], lhsT=attn[:], rhs=ws2_sb[:],
                             start=False, stop=True)
            score_sb = sb.tile([1, 1], f32, tag="score_sb")
            nc.vector.tensor_tensor(
                out=score_sb[:], in0=s_ps[:], in1=scal_sb[:, 2:3],
                op=mybir.AluOpType.add,
            )
            nc.sync.dma_start(score_d[:], score_sb[:])

            # ---- GRU step ----
            gi_ps = ps.tile([128, 3], f32, tag="gi_ps")
            gh_ps = ps.tile([128, 3], f32, tag="gh_ps")
            for j in range(3):
                nc.tensor.matmul(
                    gi_ps[:, j:j + 1],
                    lhsT=wihT_sb[:, j * 128:(j + 1) * 128],
                    rhs=v_sb[:], start=True, stop=True,
                )
                nc.tensor.matmul(
                    gh_ps[:, j:j + 1],
                    lhsT=whhT_sb[:, j * 128:(j + 1) * 128],
                    rhs=h0_sb[:], start=True, stop=True,
                )
            gi0 = sb.tile([128, 3], f32, tag="gi0")
            nc.vector.tensor_add(out=gi0[:], in0=gi_ps[:], in1=bih3_sb[:])
            gi = sb.tile([128, 3], f32, tag="gi")
            # gi = Wihs * s + gi0
            nc.vector.scalar_tensor_tensor(
                out=gi[:], in0=wihs_sb[:], scalar=bcast[:, 2:3], in1=gi0[:],
                op0=mybir.AluOpType.mult, op1=mybir.AluOpType.add,
            )
            gh = sb.tile([128, 3], f32, tag="gh")
            nc.vector.tensor_add(out=gh[:], in0=gh_ps[:], in1=bhh3_sb[:])
            prerz = sb.tile([128, 2], f32, tag="prerz")
            nc.vector.tensor_add(out=prerz[:], in0=gi[:, 0:2], in1=gh[:, 0:2])
            rz = sb.tile([128, 2], f32, tag="rz")
            nc.scalar.activation(rz[:], prerz[:],
                                 mybir.ActivationFunctionType.Sigmoid)
            npre = sb.tile([128, 1], f32, tag="npre")
            # npre = gh_n * r + gi_n
            nc.vector.scalar_tensor_tensor(
                out=npre[:], in0=gh[:, 2:3], scalar=rz[:, 0:1], in1=gi[:, 2:3],
                op0=mybir.AluOpType.mult, op1=mybir.AluOpType.add,
            )
            ntan = sb.tile([128, 1], f32, tag="ntan")
            nc.scalar.activation(ntan[:], npre[:],
                                 mybir.ActivationFunctionType.Tanh)
            hdiff = sb.tile([128, 1], f32, tag="hdiff")
            nc.vector.tensor_sub(out=hdiff[:], in0=h0_sb[:], in1=ntan[:])
            hnew = sb.tile([128, 1], f32, tag="hnew")
            # hnew = hdiff * z + ntan  = (1-z)*n + z*h0
            nc.vector.scalar_tensor_tensor(
                out=hnew[:], in0=hdiff[:], scalar=rz[:, 1:2], in1=ntan[:],
                op0=mybir.AluOpType.mult, op1=mybir.AluOpType.add,
            )
            nc.sync.dma_start(hout_d[:], hnew[:])

            # ---- debug capture ----
            dbg = sb.tile([128, 32], f32, tag="dbg")
            nc.vector.memset(dbg[:], 0.0)
            nc.vector.tensor_copy(out=dbg[:, 0:8], in_=sel[:])
            nc.vector.tensor_copy(out=dbg[:, 8:16], in_=m8v[:])
            nc.vector.tensor_copy(out=dbg[:, 16:24], in_=hsel[:, :, HID])
            nc.vector.tensor_copy(out=dbg[:, 24:25], in_=bcast[:, 0:1])
            nc.vector.tensor_copy(out=dbg[:, 25:26], in_=num_sb[:])
            nc.vector.tensor_copy(out=dbg[:, 26:27], in_=urs[:])
            nc.vector.tensor_copy(out=dbg[:, 27:28], in_=num_tot[:])
            nc.vector.tensor_copy(out=dbg[:, 28:29], in_=u[:, 0:1])
            nc.vector.tensor_copy(out=dbg[:, 29:30], in_=hsel[:, 0, 0:1])
            nc.sync.dma_start(dbg_d[:], dbg[:])

    nc.compile()
    return nc


def _prep_in_maps(v, s, t, vs, hs, ts, W_ih, b_ih, W_hh, b_hh, W_score, b_score):
    f = np.float32
    v = np.ascontiguousarray(v, f)
    vs = np.ascontiguousarray(vs, f)
    hs_flat = np.ascontiguousarray(hs, f).reshape(N_HIST, HID)
    ts = np.ascontiguousarray(ts, f)
    W_ih = np.ascontiguousarray(W_ih, f)
    W_hh = np.ascontiguousarray(W_hh, f)
    W_score = np.ascontiguousarray(W_score, f)

    shared = {
        "vcol": v.reshape(128, 1).copy(),
        "scal": np.array(
            [[float(t[0]), float(s[0]), float(b_score[0]), 0, 0, 0, 0, 0]], f
        ),
        "Ws1": W_score[0, :128].reshape(128, 1).copy(),
        "Ws2": W_score[0, 128:].reshape(128, 1).copy(),
        "WihT": np.ascontiguousarray(W_ih[:, :128].T),
        "Wihs": np.ascontiguousarray(W_ih[:, 128].reshape(3, 128).T),
        "WhhT": np.ascontiguousarray(W_hh.T),
        "bih3": np.ascontiguousarray(np.asarray(b_ih, f).reshape(3, 128).T),
        "bhh3": np.ascontiguousarray(np.asarray(b_hh, f).reshape(3, 128).T),
        "h0": hs_flat[-1].reshape(128, 1).copy(),
        "iota_p": np.arange(128, dtype=f).reshape(128, 1),
    }
    in_maps = []
    for c in range(NCORES):
        sl = slice(c * SHARD, (c + 1) * SHARD)
        vsT = np.zeros((128, PADN), f)
        vsT[:, :SHARD] = vs[sl].T
        # pad columns dot v to ~-1e9 so they can never enter any top-k
        padvec = v * np.float32(-1e9 / float(np.dot(v, v)))
        vsT[:, SHARD:] = padvec.reshape(128, 1)
        hs_aug = np.zeros((AUGN, AUGW), f)
        hs_aug[:SHARD, :HID] = hs_flat[sl]
        hs_aug[:SHARD, HID] = ts[sl]
        m = dict(shared)
        m["vsT"] = vsT
        m["hs_aug"] = hs_aug
        in_maps.append(m)
    return in_maps


def run_on_hw(in_maps, trace=False):
    from concourse.bass_utils import run_bass_kernel_spmd

    if "nc" not in _CACHE:
        _CACHE["nc"] = _build_program()
    nc = _CACHE["nc"]
    return run_bass_kernel_spmd(
        nc, in_maps, core_ids=list(range(NCORES)), trace=trace,
    )


def kernel(v, s, t, vs, hs, ts, W_ih, b_ih, W_hh, b_hh, W_score, b_score):
    in_maps = _prep_in_maps(
        v, s, t, vs, hs, ts, W_ih, b_ih, W_hh, b_hh, W_score, b_score
    )
    res = run_on_hw(in_maps).results
    score = np.asarray(res[0]["score_out"], np.float32).reshape(1, 1)
    h_new = np.asarray(res[0]["h_out"], np.float32).reshape(1, 1, HID)
    return score, h_new
